# revision 3
# baseline (speedup 1.0000x reference)
"""Trainium2 Bass kernel v3 for the Mobius-addition broadcast problem.

out[m, n, :] = a[m,n]*B[n, :] + b[m,n]*x[m, :]
  a = coefB/denom, b = coefx/denom (rec = 1/denom folded into both planes).

fp16 output path (rel tol 2e-2 allows it): halves the output-DMA floor.
Per m (natural layout, psum [128 n-in-block, 8 nb x 128 d]):
  - 8 rank-1 outer matmuls b16[m,nb-seg] (x) x16[m]  (PE, fp16)
  - cb16 = a_col (.) B16 per nb: 8 small tensor_scalar ops split DVE/Pool
  - add cb16: eye-matmul accumulate (PE) / tt_add (DVE) / stt-add (Pool)
  - evacuate psum -> fp16 SBUF: ScalarE copy / DVE copy / direct f32 DMA
  - one DMA per m, fp16 (f32 for the direct-psum rows)
Engine mix per 50 rows tuned so every engine sits just under the DMA
roofline (~800 ns/row).
"""

import sys
from contextlib import ExitStack

import numpy as np

sys.path.insert(0, "/opt/trn_rl_repo")

import concourse.bacc as bacc  # noqa: E402
import concourse.tile as tile  # noqa: E402
from concourse import mybir  # noqa: E402

N, M, D = 1024, 2048, 128
NCORES = 8
MC = M // NCORES  # 256 rows of x per core
NBS = N // 128    # 8 n-blocks
MBS = MC // 128   # 2 m-blocks
F32 = mybir.dt.float32
F16 = mybir.dt.float16
ALU = mybir.AluOpType
ACT = mybir.ActivationFunctionType

# per-m type pattern: es=eye+scal, ev=eye+dve, ts=scal+dve_tt, us=scal+pool_stt,
# ew=eye+direct f32 DMA from psum
PSUM16 = False  # fp16 PSUM matmul output is rejected by bass (fp32 only)
# pair-aligned types (cb tiles and output DMAs are shared per pair)
_PAIR_COUNTS = [("es", 37), ("tv", 11)]
_COUNTS = [(t, 2 * c) for t, c in _PAIR_COUNTS]


def _mk_pattern():
    # largest-remainder interleave over PAIRS so types spread evenly and
    # each even/odd pair shares a type
    total = sum(c for _, c in _PAIR_COUNTS)
    slots = []
    fills = {n: 0 for n, _ in _PAIR_COUNTS}
    for i in range(total):
        best, bn = None, None
        for n, c in _PAIR_COUNTS:
            want = (c / total) * (i + 1) - fills[n]
            if best is None or want > best:
                best, bn = want, n
        slots += [bn, bn]
        fills[bn] += 1
    return slots


PATTERN = _mk_pattern()
KPAIRS = 3  # cb production lookahead, in pairs


def _type_of(m):
    return PATTERN[m % len(PATTERN)]


def _body(ctx, tc, out16_d, out32_d, bt16_d, b16_d, xt16_d, x16_d, eye_d,
          bnat_d):
    nc = tc.nc
    consts = ctx.enter_context(tc.tile_pool(name="consts", bufs=1))

    # ---- static inputs ----
    bt16 = consts.tile([128, N], F16)      # B^T [d, n]
    nc.sync.dma_start(bt16[:], bt16_d[:, :])
    b16b = consts.tile([128, N], F16)      # B block layout [n-in-blk, (nb,d)]
    nc.sync.dma_start(b16b[:], b16_d[:, :])
    xt16 = consts.tile([128, MC], F16)     # x^T [d, m]
    nc.sync.dma_start(xt16[:], xt16_d[:, :])
    x16r = consts.tile([1, MC * D], F16)   # x rows flattened on partition 0
    nc.sync.dma_start(x16r[:], x16_d[:, :])
    eye16 = consts.tile([128, 128], F16)
    nc.sync.dma_start(eye16[:], eye_d[:, :])

    ones_col = consts.tile([128, 1], F16)
    nc.vector.memset(ones_col[:], 1.0)
    ones_row = consts.tile([1, 128], F16)
    nc.vector.memset(ones_row[:], 1.0)
    # touch the activation table at t=0 so its 1.3us load overlaps input DMAs
    warm = consts.tile([1, 1], F32)
    nc.vector.memset(warm[:], 0.0)
    nc.scalar.copy(warm[:], warm[:])

    # ---- persistent planes ----
    aT = consts.tile([128, NBS * MC], F32)   # a transposed [n-part, nb*MC+m]
    b16n = consts.tile([128, MBS * N], F16)  # b natural [m-part, mb*N+n]
    nB16 = consts.tile([1, N], F16)
    nx16 = consts.tile([1, MC], F16)
    cfx16 = consts.tile([1, N], F16)         # coefx = 1-nB
    cfxb16 = consts.tile([128, N], F16)      # coefx broadcast across partitions

    with ExitStack() as pctx:
        ptmp = pctx.enter_context(tc.tile_pool(name="ptmp", bufs=2))
        prow = pctx.enter_context(tc.tile_pool(name="prow", bufs=1, space="PSUM"))
        ppl = pctx.enter_context(tc.tile_pool(name="ppl", bufs=2, space="PSUM"))
        pbp = pctx.enter_context(tc.tile_pool(name="pbp", bufs=1, space="PSUM"))
        pnat = pctx.enter_context(tc.tile_pool(name="pnat", bufs=1, space="PSUM"))

        xt2 = consts.tile([128, MC], F16)    # 2*x^T
        nc.vector.tensor_scalar_mul(xt2[:], xt16[:], 2.0)
        btsq = ptmp.tile([128, N], F16, tag="btsq")
        nc.vector.tensor_tensor(out=btsq[:], in0=bt16[:], in1=bt16[:], op=ALU.mult)
        xtsq = ptmp.tile([128, MC], F16, tag="xtsq")
        nc.vector.tensor_tensor(out=xtsq[:], in0=xt16[:], in1=xt16[:], op=ALU.mult)

        # nB = |B_n|^2, nx = |x_m|^2 (column-sum via ones matmul)
        for h in range(2):
            pr = prow.tile([1, 512], F32, tag="pr")
            nc.tensor.matmul(pr[:], ones_col[:], btsq[:, h * 512:(h + 1) * 512],
                             start=True, stop=True)
            nc.scalar.copy(nB16[:, h * 512:(h + 1) * 512], pr[:])
        prx = prow.tile([1, 512], F32, tag="pr")
        nc.tensor.matmul(prx[:, :MC], ones_col[:], xtsq[:], start=True, stop=True)
        nc.scalar.copy(nx16[:], prx[:, :MC])

        # coefx = 1 - nB ; broadcast across partitions via rank-1 matmul
        nc.vector.tensor_scalar(cfx16[:], nB16[:], -1.0, 1.0,
                                op0=ALU.mult, op1=ALU.add)
        for h in range(2):
            pb = pbp.tile([128, 512], F32, tag="pb")
            nc.tensor.matmul(pb[:], ones_row[:], cfx16[:, h * 512:(h + 1) * 512],
                             start=True, stop=True)
            nc.scalar.copy(cfxb16[:, h * 512:(h + 1) * 512], pb[:])

        # ---- transposed pass: aT[n-part, m] per n-block ----
        for nb in range(NBS):
            sl = slice(nb * 128, (nb + 1) * 128)
            pp = ppl.tile([128, 2 * MC], F32, tag="pp")
            ps1, ps2 = pp[:, :MC], pp[:, MC:]
            nc.tensor.matmul(ps1, bt16[:, sl], xt2[:], start=True, stop=False)
            nc.tensor.matmul(ps1, nB16[:, sl], nx16[:], start=False, stop=True)
            nc.tensor.matmul(ps2, bt16[:, sl], xt2[:], start=True, stop=False)
            nc.tensor.matmul(ps2, ones_row[:], nx16[:], start=False, stop=True)
            dencf = ptmp.tile([128, 2 * MC], F32, tag="dencf")
            nc.scalar.activation(dencf[:], pp[:], ACT.Copy, bias=1.0, scale=1.0)
            rec = ptmp.tile([128, MC], F32, tag="rec")
            nc.vector.reciprocal(rec[:], dencf[:, :MC])
            nc.gpsimd.tensor_tensor(out=aT[:, nb * MC:(nb + 1) * MC],
                                    in0=dencf[:, MC:], in1=rec[:], op=ALU.mult)

        # ---- natural pass first: b16n[m-part, n] per m-block (its DRAM
        # roundtrip + staging overlaps the transposed pass below) ----
        for mb in range(MBS):
            msl = slice(mb * 128, (mb + 1) * 128)
            psn = pnat.tile([128, N], F32, tag="psn")
            for h in range(2):
                hsl = slice(h * 512, (h + 1) * 512)
                nc.tensor.matmul(psn[:, hsl], xt2[:, msl], bt16[:, hsl],
                                 start=True, stop=False)
                nc.tensor.matmul(psn[:, hsl], nx16[:, msl], nB16[:, hsl],
                                 start=False, stop=True)
            dnat = ptmp.tile([128, N], F32, tag="dnat")
            nc.scalar.activation(dnat[:], psn[:], ACT.Copy, bias=1.0, scale=1.0)
            rnat = ptmp.tile([128, N], F16, tag="rnat")
            with nc.allow_low_precision(reason="fp16 rec within 2e-2 tol"):
                nc.vector.reciprocal(rnat[:], dnat[:])
            nc.gpsimd.tensor_tensor(out=b16n[:, mb * N:(mb + 1) * N],
                                    in0=rnat[:], in1=cfxb16[:], op=ALU.mult)
            # stage b rows to DRAM so the main loop can reload them at
            # partition base 0 (matmul lhsT constraint)
            nc.sync.dma_start(
                bnat_d[mb:mb + 1, :].rearrange("one (p n) -> (one p) n", p=128),
                b16n[:, mb * N:(mb + 1) * N])

    # ---- main loop ----
    PSDT = F16 if PSUM16 else F32
    pm = ctx.enter_context(tc.tile_pool(name="pm", bufs=4, space="PSUM"))
    cbp = ctx.enter_context(tc.tile_pool(name="cbp", bufs=KPAIRS + 2))
    t16p = ctx.enter_context(tc.tile_pool(name="t16p", bufs=2))
    otp = ctx.enter_context(tc.tile_pool(name="otp", bufs=4))
    bstp = ctx.enter_context(tc.tile_pool(name="bstp", bufs=3))

    CHB = 8  # staged b rows per chunk
    cb_tiles = {}
    bst_tiles = {}

    def stage_b(c):
        m0 = c * CHB
        mb, mr0 = divmod(m0, 128)
        bst = bstp.tile([1, CHB * N], F16, tag="bst")
        # issue on the Activation DGE queue so output DMAs (SP queue) never
        # block staging
        nc.scalar.dma_start(
            bst[:], bnat_d[mb:mb + 1, mr0 * N:(mr0 + CHB) * N])
        bst_tiles[c] = bst

    def produce_pair(pp):
        # cb for rows 2*pp and 2*pp+1 into one pair-wide tile
        cb2 = cbp.tile([128, 2 * N], F16, tag="cb", name=f"cb{pp % (KPAIRS + 2)}")
        for half in range(2):
            mp = 2 * pp + half
            nd = 6 if mp % 2 == 0 else 5  # block-ops on DVE; rest on Pool
            for nb in range(NBS):
                sl = slice(half * N + nb * 128, half * N + (nb + 1) * 128)
                eng = nc.vector if nb < nd else nc.gpsimd
                eng.tensor_scalar_mul(cb2[:, sl], b16b[:, nb * 128:(nb + 1) * 128],
                                      aT[:, nb * MC + mp:nb * MC + mp + 1])
        cb_tiles[pp] = cb2

    for pp in range(min(KPAIRS, MC // 2)):
        produce_pair(pp)
    stage_b(0)
    stage_b(1)

    for m in range(MC):
        if m % 2 == 0 and m // 2 + KPAIRS < MC // 2:
            produce_pair(m // 2 + KPAIRS)
        if m % CHB == 0 and (m // CHB) + 2 < MC // CHB:
            stage_b((m // CHB) + 2)
        typ = _type_of(m)
        cb2 = cb_tiles[m // 2]
        cb = cb2[:, (m % 2) * N:(m % 2 + 1) * N]
        bst = bst_tiles[m // CHB]
        boff = (m % CHB) * N
        if m % 2 == 0:
            ot2 = otp.tile([128, 2 * N], F16, tag="ot", name=f"ot{(m // 2) % 4}")
        osl = ot2[:, (m % 2) * N:(m % 2 + 1) * N]
        ps = pm.tile([128, N], PSDT, tag="pm", name=f"pm{m % 4}")
        eye_add = typ == "es"
        for nb in range(NBS):
            qsl = slice(nb * 128, (nb + 1) * 128)
            nc.tensor.matmul(
                ps[:, qsl],
                bst[:, boff + nb * 128: boff + (nb + 1) * 128],
                x16r[:, m * D:(m + 1) * D],
                start=True, stop=not eye_add)
            if eye_add:
                # close each quadrant's accumulation group before the next
                # opens (PSUM group state is bank-scoped)
                nc.tensor.matmul(ps[:, qsl], eye16[:], cb[:, qsl],
                                 start=False, stop=True)
        if m % CHB == CHB - 1:
            bst_tiles.pop(m // CHB)
        if typ == "es":
            nc.scalar.copy(osl, ps[:])
        elif typ == "tv":
            nc.vector.scalar_tensor_tensor(
                out=osl, in0=ps[:], scalar=1.0, in1=cb[:],
                op0=ALU.mult, op1=ALU.add)
        else:  # ts
            t16 = t16p.tile([128, N], F16, tag="t16")
            nc.scalar.copy(t16[:], ps[:])
            nc.vector.tensor_tensor(out=osl, in0=t16[:], in1=cb[:],
                                    op=ALU.add)
        if m % 2 == 1:
            cb_tiles.pop(m // 2)
            nc.sync.dma_start(
                out16_d[m - 1:m + 1, :, :].rearrange("two p f -> p two f"),
                ot2[:])


def build_program(mc=MC, n=N):
    nc = bacc.Bacc("TRN2", target_bir_lowering=False, debug=False,
                   num_devices=NCORES)
    bt16_d = nc.dram_tensor("bt16_in", [D, n], F16, kind="ExternalInput").ap()
    b16_d = nc.dram_tensor("b16_in", [128, n], F16, kind="ExternalInput").ap()
    xt16_d = nc.dram_tensor("xt16_in", [D, mc], F16, kind="ExternalInput").ap()
    x16_d = nc.dram_tensor("x16_in", [1, mc * D], F16,
                           kind="ExternalInput").ap()
    eye_d = nc.dram_tensor("eye_in", [128, 128], F16,
                           kind="ExternalInput").ap()
    out16_d = nc.dram_tensor("out16", [mc, 128, n], F16,
                             kind="ExternalOutput").ap()
    bnat_d = nc.dram_tensor("bnat_scratch", [MBS, 128 * n], F16).ap()
    with tile.TileContext(nc) as tc:
        with ExitStack() as ctx:
            _body(ctx, tc, out16_d, None, bt16_d, b16_d, xt16_d, x16_d,
                  eye_d, bnat_d)
    nc.compile()
    return nc


_NC_CACHE = None


def _get_nc():
    global _NC_CACHE
    if _NC_CACHE is None:
        _NC_CACHE = build_program()
    return _NC_CACHE


def make_in_maps(B, x):
    B = np.ascontiguousarray(np.asarray(B, dtype=np.float32))
    x = np.ascontiguousarray(np.asarray(x, dtype=np.float32))
    bt16 = np.ascontiguousarray(B.T.astype(np.float16))
    b16blk = np.ascontiguousarray(
        B.reshape(NBS, 128, D).transpose(1, 0, 2).reshape(128, N)
        .astype(np.float16))
    eye16 = np.eye(128, dtype=np.float16)
    in_maps = []
    for c in range(NCORES):
        xs = x[c * MC:(c + 1) * MC]
        in_maps.append({
            "bt16_in": bt16,
            "b16_in": b16blk,
            "xt16_in": np.ascontiguousarray(xs.T.astype(np.float16)),
            "x16_in": np.ascontiguousarray(
                xs.astype(np.float16).reshape(1, MC * D)),
            "eye_in": eye16,
        })
    return in_maps


def kernel(B, x):
    from concourse.bass_utils import run_bass_kernel_spmd
    nc = _get_nc()
    in_maps = make_in_maps(B, x)
    res = run_bass_kernel_spmd(nc, in_maps, list(range(NCORES)))
    outs = []
    for c in range(NCORES):
        full = np.asarray(res.results[c]["out16"]).astype(np.float32)
        # [mc, p, nb, d] -> [mc, nb, p, d] -> [mc, n, d]
        full = full.reshape(MC, 128, NBS, D).transpose(0, 2, 1, 3)
        outs.append(full.reshape(MC, N, D))
    return np.concatenate(outs, axis=0)


# revision 4
# speedup vs baseline: 1.0096x; 1.0096x over previous
"""Trainium2 Bass kernel v3 for the Mobius-addition broadcast problem.

out[m, n, :] = a[m,n]*B[n, :] + b[m,n]*x[m, :]
  a = coefB/denom, b = coefx/denom (rec = 1/denom folded into both planes).

fp16 output path (rel tol 2e-2 allows it): halves the output-DMA floor.
Per m (natural layout, psum [128 n-in-block, 8 nb x 128 d]):
  - 8 rank-1 outer matmuls b16[m,nb-seg] (x) x16[m]  (PE, fp16)
  - cb16 = a_col (.) B16 per nb: 8 small tensor_scalar ops split DVE/Pool
  - add cb16: eye-matmul accumulate (PE) / tt_add (DVE) / stt-add (Pool)
  - evacuate psum -> fp16 SBUF: ScalarE copy / DVE copy / direct f32 DMA
  - one DMA per m, fp16 (f32 for the direct-psum rows)
Engine mix per 50 rows tuned so every engine sits just under the DMA
roofline (~800 ns/row).
"""

import sys
from contextlib import ExitStack

import numpy as np

sys.path.insert(0, "/opt/trn_rl_repo")

import concourse.bacc as bacc  # noqa: E402
import concourse.tile as tile  # noqa: E402
from concourse import mybir  # noqa: E402

N, M, D = 1024, 2048, 128
NCORES = 8
MC = M // NCORES  # 256 rows of x per core
NBS = N // 128    # 8 n-blocks
MBS = MC // 128   # 2 m-blocks
F32 = mybir.dt.float32
F16 = mybir.dt.float16
ALU = mybir.AluOpType
ACT = mybir.ActivationFunctionType

# per-m type pattern: es=eye+scal, ev=eye+dve, ts=scal+dve_tt, us=scal+pool_stt,
# ew=eye+direct f32 DMA from psum
PSUM16 = False  # fp16 PSUM matmul output is rejected by bass (fp32 only)
# pair-aligned types (cb tiles and output DMAs are shared per pair)
_PAIR_COUNTS = [("es", 37), ("tv", 11)]
_COUNTS = [(t, 2 * c) for t, c in _PAIR_COUNTS]


def _mk_pattern():
    # largest-remainder interleave over PAIRS so types spread evenly and
    # each even/odd pair shares a type
    total = sum(c for _, c in _PAIR_COUNTS)
    slots = []
    fills = {n: 0 for n, _ in _PAIR_COUNTS}
    for i in range(total):
        best, bn = None, None
        for n, c in _PAIR_COUNTS:
            want = (c / total) * (i + 1) - fills[n]
            if best is None or want > best:
                best, bn = want, n
        slots += [bn, bn]
        fills[bn] += 1
    return slots


PATTERN = _mk_pattern()
KPAIRS = 1  # cb production lookahead, in pairs


def _type_of(m):
    return PATTERN[m % len(PATTERN)]


def _body(ctx, tc, out16_d, out32_d, bt16_d, b16_d, xt16_d, x16_d, eye_d,
          bnat_d):
    nc = tc.nc
    consts = ctx.enter_context(tc.tile_pool(name="consts", bufs=1))

    # ---- static inputs ----
    bt16 = consts.tile([128, N], F16)      # B^T [d, n]
    nc.sync.dma_start(bt16[:], bt16_d[:, :])
    b16b = consts.tile([128, N], F16)      # B block layout [n-in-blk, (nb,d)]
    nc.sync.dma_start(b16b[:], b16_d[:, :])
    xt16 = consts.tile([128, MC], F16)     # x^T [d, m]
    nc.sync.dma_start(xt16[:], xt16_d[:, :])
    x16r = consts.tile([1, MC * D], F16)   # x rows flattened on partition 0
    nc.sync.dma_start(x16r[:], x16_d[:, :])
    eye16 = consts.tile([128, 128], F16)
    nc.sync.dma_start(eye16[:], eye_d[:, :])

    ones_col = consts.tile([128, 1], F16)
    nc.vector.memset(ones_col[:], 1.0)
    ones_row = consts.tile([1, 128], F16)
    nc.vector.memset(ones_row[:], 1.0)
    # touch the activation table at t=0 so its 1.3us load overlaps input DMAs
    warm = consts.tile([1, 1], F32)
    nc.vector.memset(warm[:], 0.0)
    nc.scalar.copy(warm[:], warm[:])

    # ---- persistent planes ----
    aT = consts.tile([128, NBS * MC], F32)   # a transposed [n-part, nb*MC+m]
    b16n = consts.tile([128, MBS * N], F16)  # b natural [m-part, mb*N+n]
    nB16 = consts.tile([1, N], F16)
    nx16 = consts.tile([1, MC], F16)
    cfx16 = consts.tile([1, N], F16)         # coefx = 1-nB
    cfxb16 = consts.tile([128, N], F16)      # coefx broadcast across partitions

    with ExitStack() as pctx:
        ptmp = pctx.enter_context(tc.tile_pool(name="ptmp", bufs=2))
        prow = pctx.enter_context(tc.tile_pool(name="prow", bufs=1, space="PSUM"))
        ppl = pctx.enter_context(tc.tile_pool(name="ppl", bufs=2, space="PSUM"))
        pbp = pctx.enter_context(tc.tile_pool(name="pbp", bufs=1, space="PSUM"))
        pnat = pctx.enter_context(tc.tile_pool(name="pnat", bufs=1, space="PSUM"))

        xt2 = consts.tile([128, MC], F16)    # 2*x^T
        nc.vector.tensor_scalar_mul(xt2[:], xt16[:], 2.0)
        btsq = ptmp.tile([128, N], F16, tag="btsq")
        nc.vector.tensor_tensor(out=btsq[:], in0=bt16[:], in1=bt16[:], op=ALU.mult)
        xtsq = ptmp.tile([128, MC], F16, tag="xtsq")
        nc.vector.tensor_tensor(out=xtsq[:], in0=xt16[:], in1=xt16[:], op=ALU.mult)

        # nB = |B_n|^2, nx = |x_m|^2 (column-sum via ones matmul)
        for h in range(2):
            pr = prow.tile([1, 512], F32, tag="pr")
            nc.tensor.matmul(pr[:], ones_col[:], btsq[:, h * 512:(h + 1) * 512],
                             start=True, stop=True)
            nc.vector.tensor_copy(nB16[:, h * 512:(h + 1) * 512], pr[:])
        prx = prow.tile([1, 512], F32, tag="pr")
        nc.tensor.matmul(prx[:, :MC], ones_col[:], xtsq[:], start=True, stop=True)
        nc.vector.tensor_copy(nx16[:], prx[:, :MC])

        # coefx = 1 - nB ; broadcast across partitions via rank-1 matmul
        nc.vector.tensor_scalar(cfx16[:], nB16[:], -1.0, 1.0,
                                op0=ALU.mult, op1=ALU.add)
        for h in range(2):
            pb = pbp.tile([128, 512], F32, tag="pb")
            nc.tensor.matmul(pb[:], ones_row[:], cfx16[:, h * 512:(h + 1) * 512],
                             start=True, stop=True)
            nc.vector.tensor_copy(cfxb16[:, h * 512:(h + 1) * 512], pb[:])

        # ---- transposed pass: aT[n-part, m] per n-block ----
        for nb in range(NBS):
            sl = slice(nb * 128, (nb + 1) * 128)
            pp = ppl.tile([128, 2 * MC], F32, tag="pp")
            ps1, ps2 = pp[:, :MC], pp[:, MC:]
            nc.tensor.matmul(ps1, bt16[:, sl], xt2[:], start=True, stop=False)
            nc.tensor.matmul(ps1, nB16[:, sl], nx16[:], start=False, stop=True)
            nc.tensor.matmul(ps2, bt16[:, sl], xt2[:], start=True, stop=False)
            nc.tensor.matmul(ps2, ones_row[:], nx16[:], start=False, stop=True)
            dencf = ptmp.tile([128, 2 * MC], F32, tag="dencf")
            nc.scalar.activation(dencf[:], pp[:], ACT.Copy, bias=1.0, scale=1.0)
            rec = ptmp.tile([128, MC], F32, tag="rec")
            nc.vector.reciprocal(rec[:], dencf[:, :MC])
            nc.gpsimd.tensor_tensor(out=aT[:, nb * MC:(nb + 1) * MC],
                                    in0=dencf[:, MC:], in1=rec[:], op=ALU.mult)

        # ---- natural pass first: b16n[m-part, n] per m-block (its DRAM
        # roundtrip + staging overlaps the transposed pass below) ----
        for mb in range(MBS):
            msl = slice(mb * 128, (mb + 1) * 128)
            psn = pnat.tile([128, N], F32, tag="psn")
            for h in range(2):
                hsl = slice(h * 512, (h + 1) * 512)
                nc.tensor.matmul(psn[:, hsl], xt2[:, msl], bt16[:, hsl],
                                 start=True, stop=False)
                nc.tensor.matmul(psn[:, hsl], nx16[:, msl], nB16[:, hsl],
                                 start=False, stop=True)
            dnat = ptmp.tile([128, N], F32, tag="dnat")
            nc.scalar.activation(dnat[:], psn[:], ACT.Copy, bias=1.0, scale=1.0)
            rnat = ptmp.tile([128, N], F16, tag="rnat")
            with nc.allow_low_precision(reason="fp16 rec within 2e-2 tol"):
                nc.vector.reciprocal(rnat[:], dnat[:])
            nc.gpsimd.tensor_tensor(out=b16n[:, mb * N:(mb + 1) * N],
                                    in0=rnat[:], in1=cfxb16[:], op=ALU.mult)
            # stage b rows to DRAM so the main loop can reload them at
            # partition base 0 (matmul lhsT constraint)
            nc.sync.dma_start(
                bnat_d[mb:mb + 1, :].rearrange("one (p n) -> (one p) n", p=128),
                b16n[:, mb * N:(mb + 1) * N])

    # ---- main loop ----
    PSDT = F16 if PSUM16 else F32
    pm = ctx.enter_context(tc.tile_pool(name="pm", bufs=4, space="PSUM"))
    cbp = ctx.enter_context(tc.tile_pool(name="cbp", bufs=KPAIRS + 2))
    t16p = ctx.enter_context(tc.tile_pool(name="t16p", bufs=2))
    otp = ctx.enter_context(tc.tile_pool(name="otp", bufs=4))
    bstp = ctx.enter_context(tc.tile_pool(name="bstp", bufs=3))

    CHB = 8  # staged b rows per chunk
    cb_tiles = {}
    bst_tiles = {}

    def stage_b(c):
        m0 = c * CHB
        mb, mr0 = divmod(m0, 128)
        bst = bstp.tile([1, CHB * N], F16, tag="bst")
        # issue on the Activation DGE queue so output DMAs (SP queue) never
        # block staging
        nc.scalar.dma_start(
            bst[:], bnat_d[mb:mb + 1, mr0 * N:(mr0 + CHB) * N])
        bst_tiles[c] = bst

    def produce_pair(pp):
        # cb for rows 2*pp and 2*pp+1 into one pair-wide tile
        cb2 = cbp.tile([128, 2 * N], F16, tag="cb", name=f"cb{pp % (KPAIRS + 2)}")
        for half in range(2):
            mp = 2 * pp + half
            nd = 6 if mp % 2 == 0 else 5  # block-ops on DVE; rest on Pool
            for nb in range(NBS):
                sl = slice(half * N + nb * 128, half * N + (nb + 1) * 128)
                eng = nc.vector if nb < nd else nc.gpsimd
                eng.tensor_scalar_mul(cb2[:, sl], b16b[:, nb * 128:(nb + 1) * 128],
                                      aT[:, nb * MC + mp:nb * MC + mp + 1])
        cb_tiles[pp] = cb2

    for pp in range(min(KPAIRS, MC // 2)):
        produce_pair(pp)
    stage_b(0)
    stage_b(1)

    for m in range(MC):
        if m % 2 == 0 and m // 2 + KPAIRS < MC // 2:
            produce_pair(m // 2 + KPAIRS)
        if m % CHB == 0 and (m // CHB) + 2 < MC // CHB:
            stage_b((m // CHB) + 2)
        typ = _type_of(m)
        cb2 = cb_tiles[m // 2]
        cb = cb2[:, (m % 2) * N:(m % 2 + 1) * N]
        bst = bst_tiles[m // CHB]
        boff = (m % CHB) * N
        if m % 2 == 0:
            ot2 = otp.tile([128, 2 * N], F16, tag="ot", name=f"ot{(m // 2) % 4}")
        osl = ot2[:, (m % 2) * N:(m % 2 + 1) * N]
        ps = pm.tile([128, N], PSDT, tag="pm", name=f"pm{m % 4}")
        eye_add = typ == "es"
        for nb in range(NBS):
            qsl = slice(nb * 128, (nb + 1) * 128)
            nc.tensor.matmul(
                ps[:, qsl],
                bst[:, boff + nb * 128: boff + (nb + 1) * 128],
                x16r[:, m * D:(m + 1) * D],
                start=True, stop=not eye_add)
            if eye_add:
                # close each quadrant's accumulation group before the next
                # opens (PSUM group state is bank-scoped)
                nc.tensor.matmul(ps[:, qsl], eye16[:], cb[:, qsl],
                                 start=False, stop=True)
        if m % CHB == CHB - 1:
            bst_tiles.pop(m // CHB)
        if typ == "es":
            nc.scalar.copy(osl, ps[:])
        elif typ == "tv":
            nc.vector.scalar_tensor_tensor(
                out=osl, in0=ps[:], scalar=1.0, in1=cb[:],
                op0=ALU.mult, op1=ALU.add)
        else:  # ts
            t16 = t16p.tile([128, N], F16, tag="t16")
            nc.scalar.copy(t16[:], ps[:])
            nc.vector.tensor_tensor(out=osl, in0=t16[:], in1=cb[:],
                                    op=ALU.add)
        if m % 2 == 1:
            cb_tiles.pop(m // 2)
            nc.sync.dma_start(
                out16_d[m - 1:m + 1, :, :].rearrange("two p f -> p two f"),
                ot2[:])


def build_program(mc=MC, n=N):
    nc = bacc.Bacc("TRN2", target_bir_lowering=False, debug=False,
                   num_devices=NCORES)
    bt16_d = nc.dram_tensor("bt16_in", [D, n], F16, kind="ExternalInput").ap()
    b16_d = nc.dram_tensor("b16_in", [128, n], F16, kind="ExternalInput").ap()
    xt16_d = nc.dram_tensor("xt16_in", [D, mc], F16, kind="ExternalInput").ap()
    x16_d = nc.dram_tensor("x16_in", [1, mc * D], F16,
                           kind="ExternalInput").ap()
    eye_d = nc.dram_tensor("eye_in", [128, 128], F16,
                           kind="ExternalInput").ap()
    out16_d = nc.dram_tensor("out16", [mc, 128, n], F16,
                             kind="ExternalOutput").ap()
    bnat_d = nc.dram_tensor("bnat_scratch", [MBS, 128 * n], F16).ap()
    with tile.TileContext(nc) as tc:
        with ExitStack() as ctx:
            _body(ctx, tc, out16_d, None, bt16_d, b16_d, xt16_d, x16_d,
                  eye_d, bnat_d)
    nc.compile()
    return nc


_NC_CACHE = None


def _get_nc():
    global _NC_CACHE
    if _NC_CACHE is None:
        _NC_CACHE = build_program()
    return _NC_CACHE


def make_in_maps(B, x):
    B = np.ascontiguousarray(np.asarray(B, dtype=np.float32))
    x = np.ascontiguousarray(np.asarray(x, dtype=np.float32))
    bt16 = np.ascontiguousarray(B.T.astype(np.float16))
    b16blk = np.ascontiguousarray(
        B.reshape(NBS, 128, D).transpose(1, 0, 2).reshape(128, N)
        .astype(np.float16))
    eye16 = np.eye(128, dtype=np.float16)
    in_maps = []
    for c in range(NCORES):
        xs = x[c * MC:(c + 1) * MC]
        in_maps.append({
            "bt16_in": bt16,
            "b16_in": b16blk,
            "xt16_in": np.ascontiguousarray(xs.T.astype(np.float16)),
            "x16_in": np.ascontiguousarray(
                xs.astype(np.float16).reshape(1, MC * D)),
            "eye_in": eye16,
        })
    return in_maps


def kernel(B, x):
    from concourse.bass_utils import run_bass_kernel_spmd
    nc = _get_nc()
    in_maps = make_in_maps(B, x)
    res = run_bass_kernel_spmd(nc, in_maps, list(range(NCORES)))
    outs = []
    for c in range(NCORES):
        full = np.asarray(res.results[c]["out16"]).astype(np.float32)
        # [mc, p, nb, d] -> [mc, nb, p, d] -> [mc, n, d]
        full = full.reshape(MC, 128, NBS, D).transpose(0, 2, 1, 3)
        outs.append(full.reshape(MC, N, D))
    return np.concatenate(outs, axis=0)


# revision 5
# speedup vs baseline: 1.0497x; 1.0398x over previous
"""Trainium2 Bass kernel v3 for the Mobius-addition broadcast problem.

out[m, n, :] = a[m,n]*B[n, :] + b[m,n]*x[m, :]
  a = coefB/denom, b = coefx/denom (rec = 1/denom folded into both planes).

fp16 output path (rel tol 2e-2 allows it): halves the output-DMA floor.
Per m (natural layout, psum [128 n-in-block, 8 nb x 128 d]):
  - 8 rank-1 outer matmuls b16[m,nb-seg] (x) x16[m]  (PE, fp16)
  - cb16 = a_col (.) B16 per nb: 8 small tensor_scalar ops split DVE/Pool
  - add cb16: eye-matmul accumulate (PE) / tt_add (DVE) / stt-add (Pool)
  - evacuate psum -> fp16 SBUF: ScalarE copy / DVE copy / direct f32 DMA
  - one DMA per m, fp16 (f32 for the direct-psum rows)
Engine mix per 50 rows tuned so every engine sits just under the DMA
roofline (~800 ns/row).
"""

import sys
from contextlib import ExitStack

import numpy as np

sys.path.insert(0, "/opt/trn_rl_repo")

import concourse.bacc as bacc  # noqa: E402
import concourse.tile as tile  # noqa: E402
from concourse import mybir  # noqa: E402

N, M, D = 1024, 2048, 128
NCORES = 8
MC = M // NCORES  # 256 rows of x per core
NBS = N // 128    # 8 n-blocks
MBS = MC // 128   # 2 m-blocks
F32 = mybir.dt.float32
F16 = mybir.dt.float16
ALU = mybir.AluOpType
ACT = mybir.ActivationFunctionType

# per-m type pattern: es=eye+scal, ev=eye+dve, ts=scal+dve_tt, us=scal+pool_stt,
# ew=eye+direct f32 DMA from psum
PSUM16 = False  # fp16 PSUM matmul output is rejected by bass (fp32 only)
# pair-aligned types (cb tiles and output DMAs are shared per pair)
_PAIR_COUNTS = [("es", 76), ("tv", 20)]
_COUNTS = [(t, 2 * c) for t, c in _PAIR_COUNTS]


def _mk_pattern():
    # largest-remainder interleave over PAIRS so types spread evenly and
    # each even/odd pair shares a type
    total = 2 * sum(c for _, c in _PAIR_COUNTS)
    slots = []
    fills = {n: 0 for n, _ in _PAIR_COUNTS}
    for i in range(total):
        best, bn = None, None
        for n, c in _PAIR_COUNTS:
            want = (2 * c / total) * (i + 1) - fills[n]
            if best is None or want > best:
                best, bn = want, n
        slots += [bn]
        fills[bn] += 1
    return slots


PATTERN = _mk_pattern()
KPAIRS = 1  # cb production lookahead, in pairs


def _type_of(m):
    return PATTERN[m % len(PATTERN)]


def _body(ctx, tc, out16_d, out32_d, bt16_d, b16_d, xt16_d, x16_d, eye_d,
          bnat_d):
    nc = tc.nc
    consts = ctx.enter_context(tc.tile_pool(name="consts", bufs=1))

    # ---- static inputs ----
    bt16 = consts.tile([128, N], F16)      # B^T [d, n]
    nc.sync.dma_start(bt16[:], bt16_d[:, :])
    b16b = consts.tile([128, N], F16)      # B block layout [n-in-blk, (nb,d)]
    nc.sync.dma_start(b16b[:], b16_d[:, :])
    xt16 = consts.tile([128, MC], F16)     # x^T [d, m]
    nc.sync.dma_start(xt16[:], xt16_d[:, :])
    x16r = consts.tile([1, MC * D], F16)   # x rows flattened on partition 0
    nc.sync.dma_start(x16r[:], x16_d[:, :])
    eye16 = consts.tile([128, 128], F16)
    nc.sync.dma_start(eye16[:], eye_d[:, :])

    ones_col = consts.tile([128, 1], F16)
    nc.vector.memset(ones_col[:], 1.0)
    ones_row = consts.tile([1, 128], F16)
    nc.vector.memset(ones_row[:], 1.0)
    # touch the activation table at t=0 so its 1.3us load overlaps input DMAs
    warm = consts.tile([1, 1], F32)
    nc.vector.memset(warm[:], 0.0)
    nc.scalar.copy(warm[:], warm[:])

    # ---- persistent planes ----
    aT = consts.tile([128, NBS * MC], F32)   # a transposed [n-part, nb*MC+m]
    b16n = consts.tile([128, MBS * N], F16)  # b natural [m-part, mb*N+n]
    nB16 = consts.tile([1, N], F16)
    nx16 = consts.tile([1, MC], F16)
    cfx16 = consts.tile([1, N], F16)         # coefx = 1-nB
    cfxb16 = consts.tile([128, N], F16)      # coefx broadcast across partitions

    with ExitStack() as pctx:
        ptmp = pctx.enter_context(tc.tile_pool(name="ptmp", bufs=2))
        prow = pctx.enter_context(tc.tile_pool(name="prow", bufs=1, space="PSUM"))
        ppl = pctx.enter_context(tc.tile_pool(name="ppl", bufs=2, space="PSUM"))
        pbp = pctx.enter_context(tc.tile_pool(name="pbp", bufs=1, space="PSUM"))
        pnat = pctx.enter_context(tc.tile_pool(name="pnat", bufs=1, space="PSUM"))

        xt2 = consts.tile([128, MC], F16)    # 2*x^T
        nc.vector.tensor_scalar_mul(xt2[:], xt16[:], 2.0)
        btsq = ptmp.tile([128, N], F16, tag="btsq")
        nc.vector.tensor_tensor(out=btsq[:], in0=bt16[:], in1=bt16[:], op=ALU.mult)
        xtsq = ptmp.tile([128, MC], F16, tag="xtsq")
        nc.vector.tensor_tensor(out=xtsq[:], in0=xt16[:], in1=xt16[:], op=ALU.mult)

        # nB = |B_n|^2, nx = |x_m|^2 (column-sum via ones matmul)
        for h in range(2):
            pr = prow.tile([1, 512], F32, tag="pr")
            nc.tensor.matmul(pr[:], ones_col[:], btsq[:, h * 512:(h + 1) * 512],
                             start=True, stop=True)
            nc.vector.tensor_copy(nB16[:, h * 512:(h + 1) * 512], pr[:])
        prx = prow.tile([1, 512], F32, tag="pr")
        nc.tensor.matmul(prx[:, :MC], ones_col[:], xtsq[:], start=True, stop=True)
        nc.vector.tensor_copy(nx16[:], prx[:, :MC])

        # coefx = 1 - nB ; broadcast across partitions via rank-1 matmul
        nc.vector.tensor_scalar(cfx16[:], nB16[:], -1.0, 1.0,
                                op0=ALU.mult, op1=ALU.add)
        for h in range(2):
            pb = pbp.tile([128, 512], F32, tag="pb")
            nc.tensor.matmul(pb[:], ones_row[:], cfx16[:, h * 512:(h + 1) * 512],
                             start=True, stop=True)
            nc.vector.tensor_copy(cfxb16[:, h * 512:(h + 1) * 512], pb[:])

        # ---- natural pass first: b16n[m-part, n] per m-block (its DRAM
        # roundtrip + staging overlaps the transposed pass below) ----
        for mb in range(MBS):
            msl = slice(mb * 128, (mb + 1) * 128)
            psn = pnat.tile([128, N], F32, tag="psn")
            for h in range(2):
                hsl = slice(h * 512, (h + 1) * 512)
                nc.tensor.matmul(psn[:, hsl], xt2[:, msl], bt16[:, hsl],
                                 start=True, stop=False)
                nc.tensor.matmul(psn[:, hsl], nx16[:, msl], nB16[:, hsl],
                                 start=False, stop=True)
            dnat = ptmp.tile([128, N], F32, tag="dnat")
            nc.scalar.activation(dnat[:], psn[:], ACT.Copy, bias=1.0, scale=1.0)
            rnat = ptmp.tile([128, N], F16, tag="rnat")
            with nc.allow_low_precision(reason="fp16 rec within 2e-2 tol"):
                nc.vector.reciprocal(rnat[:], dnat[:])
            nc.vector.tensor_tensor(out=b16n[:, mb * N:(mb + 1) * N],
                                     in0=rnat[:], in1=cfxb16[:], op=ALU.mult)
            # stage b rows to DRAM so the main loop can reload them at
            # partition base 0 (matmul lhsT constraint)
            nc.sync.dma_start(
                bnat_d[mb:mb + 1, :].rearrange("one (p n) -> (one p) n", p=128),
                b16n[:, mb * N:(mb + 1) * N])

        # ---- transposed pass: aT[n-part, m] per n-block ----
        for nb in range(NBS):
            sl = slice(nb * 128, (nb + 1) * 128)
            pp = ppl.tile([128, 2 * MC], F32, tag="pp")
            ps1, ps2 = pp[:, :MC], pp[:, MC:]
            nc.tensor.matmul(ps1, bt16[:, sl], xt2[:], start=True, stop=False)
            nc.tensor.matmul(ps1, nB16[:, sl], nx16[:], start=False, stop=True)
            nc.tensor.matmul(ps2, bt16[:, sl], xt2[:], start=True, stop=False)
            nc.tensor.matmul(ps2, ones_row[:], nx16[:], start=False, stop=True)
            dencf = ptmp.tile([128, 2 * MC], F32, tag="dencf")
            nc.scalar.activation(dencf[:], pp[:], ACT.Copy, bias=1.0, scale=1.0)
            rec = ptmp.tile([128, MC], F32, tag="rec")
            nc.vector.reciprocal(rec[:], dencf[:, :MC])
            nc.gpsimd.tensor_tensor(out=aT[:, nb * MC:(nb + 1) * MC],
                                    in0=dencf[:, MC:], in1=rec[:], op=ALU.mult)

    # ---- main loop ----
    PSDT = F16 if PSUM16 else F32
    pm = ctx.enter_context(tc.tile_pool(name="pm", bufs=4, space="PSUM"))
    cbp = ctx.enter_context(tc.tile_pool(name="cbp", bufs=KPAIRS + 2))
    t16p = ctx.enter_context(tc.tile_pool(name="t16p", bufs=2))
    otp = ctx.enter_context(tc.tile_pool(name="otp", bufs=4))
    bstp = ctx.enter_context(tc.tile_pool(name="bstp", bufs=3))

    CHB = 8  # staged b rows per chunk
    cb_tiles = {}
    bst_tiles = {}

    def stage_b(c):
        m0 = c * CHB
        mb, mr0 = divmod(m0, 128)
        bst = bstp.tile([1, CHB * N], F16, tag="bst")
        # issue on the Activation DGE queue so output DMAs (SP queue) never
        # block staging
        nc.scalar.dma_start(
            bst[:], bnat_d[mb:mb + 1, mr0 * N:(mr0 + CHB) * N])
        bst_tiles[c] = bst

    def produce_pair(pp):
        # cb for rows 2*pp and 2*pp+1 into one pair-wide tile
        cb2 = cbp.tile([128, 2 * N], F16, tag="cb", name=f"cb{pp % (KPAIRS + 2)}")
        for half in range(2):
            mp = 2 * pp + half
            nd = 6 if mp % 2 == 0 else 5  # block-ops on DVE; rest on Pool
            for nb in range(NBS):
                sl = slice(half * N + nb * 128, half * N + (nb + 1) * 128)
                eng = nc.vector if nb < nd else nc.gpsimd
                eng.tensor_scalar_mul(cb2[:, sl], b16b[:, nb * 128:(nb + 1) * 128],
                                      aT[:, nb * MC + mp:nb * MC + mp + 1])
        cb_tiles[pp] = cb2

    for pp in range(min(KPAIRS, MC // 2)):
        produce_pair(pp)
    stage_b(0)
    stage_b(1)

    for m in range(MC):
        if m % 2 == 0 and m // 2 + KPAIRS < MC // 2:
            produce_pair(m // 2 + KPAIRS)
        if m % CHB == 0 and (m // CHB) + 2 < MC // CHB:
            stage_b((m // CHB) + 2)
        typ = _type_of(m)
        cb2 = cb_tiles[m // 2]
        cb = cb2[:, (m % 2) * N:(m % 2 + 1) * N]
        bst = bst_tiles[m // CHB]
        boff = (m % CHB) * N
        if m % 2 == 0:
            ot2 = otp.tile([128, 2 * N], F16, tag="ot", name=f"ot{(m // 2) % 4}")
        osl = ot2[:, (m % 2) * N:(m % 2 + 1) * N]
        ps = pm.tile([128, N], PSDT, tag="pm", name=f"pm{m % 4}")
        eye_add = typ == "es"
        for nb in range(NBS):
            qsl = slice(nb * 128, (nb + 1) * 128)
            nc.tensor.matmul(
                ps[:, qsl],
                bst[:, boff + nb * 128: boff + (nb + 1) * 128],
                x16r[:, m * D:(m + 1) * D],
                start=True, stop=not eye_add)
            if eye_add:
                # close each quadrant's accumulation group before the next
                # opens (PSUM group state is bank-scoped)
                nc.tensor.matmul(ps[:, qsl], eye16[:], cb[:, qsl],
                                 start=False, stop=True)
        if m % CHB == CHB - 1:
            bst_tiles.pop(m // CHB)
        if typ == "es":
            nc.scalar.copy(osl, ps[:])
        elif typ == "tv":
            nc.vector.scalar_tensor_tensor(
                out=osl, in0=ps[:], scalar=1.0, in1=cb[:],
                op0=ALU.mult, op1=ALU.add)
        else:  # ts
            t16 = t16p.tile([128, N], F16, tag="t16")
            nc.scalar.copy(t16[:], ps[:])
            nc.vector.tensor_tensor(out=osl, in0=t16[:], in1=cb[:],
                                    op=ALU.add)
        if m % 2 == 1:
            cb_tiles.pop(m // 2)
            nc.sync.dma_start(
                out16_d[m - 1:m + 1, :, :].rearrange("two p f -> p two f"),
                ot2[:])


def build_program(mc=MC, n=N):
    nc = bacc.Bacc("TRN2", target_bir_lowering=False, debug=False,
                   num_devices=NCORES)
    bt16_d = nc.dram_tensor("bt16_in", [D, n], F16, kind="ExternalInput").ap()
    b16_d = nc.dram_tensor("b16_in", [128, n], F16, kind="ExternalInput").ap()
    xt16_d = nc.dram_tensor("xt16_in", [D, mc], F16, kind="ExternalInput").ap()
    x16_d = nc.dram_tensor("x16_in", [1, mc * D], F16,
                           kind="ExternalInput").ap()
    eye_d = nc.dram_tensor("eye_in", [128, 128], F16,
                           kind="ExternalInput").ap()
    out16_d = nc.dram_tensor("out16", [mc, 128, n], F16,
                             kind="ExternalOutput").ap()
    bnat_d = nc.dram_tensor("bnat_scratch", [MBS, 128 * n], F16).ap()
    with tile.TileContext(nc) as tc:
        with ExitStack() as ctx:
            _body(ctx, tc, out16_d, None, bt16_d, b16_d, xt16_d, x16_d,
                  eye_d, bnat_d)
    nc.compile()
    return nc


_NC_CACHE = None


def _get_nc():
    global _NC_CACHE
    if _NC_CACHE is None:
        _NC_CACHE = build_program()
    return _NC_CACHE


def make_in_maps(B, x):
    B = np.ascontiguousarray(np.asarray(B, dtype=np.float32))
    x = np.ascontiguousarray(np.asarray(x, dtype=np.float32))
    bt16 = np.ascontiguousarray(B.T.astype(np.float16))
    b16blk = np.ascontiguousarray(
        B.reshape(NBS, 128, D).transpose(1, 0, 2).reshape(128, N)
        .astype(np.float16))
    eye16 = np.eye(128, dtype=np.float16)
    in_maps = []
    for c in range(NCORES):
        xs = x[c * MC:(c + 1) * MC]
        in_maps.append({
            "bt16_in": bt16,
            "b16_in": b16blk,
            "xt16_in": np.ascontiguousarray(xs.T.astype(np.float16)),
            "x16_in": np.ascontiguousarray(
                xs.astype(np.float16).reshape(1, MC * D)),
            "eye_in": eye16,
        })
    return in_maps


def kernel(B, x):
    from concourse.bass_utils import run_bass_kernel_spmd
    nc = _get_nc()
    in_maps = make_in_maps(B, x)
    res = run_bass_kernel_spmd(nc, in_maps, list(range(NCORES)))
    outs = []
    for c in range(NCORES):
        full = np.asarray(res.results[c]["out16"]).astype(np.float32)
        # [mc, p, nb, d] -> [mc, nb, p, d] -> [mc, n, d]
        full = full.reshape(MC, 128, NBS, D).transpose(0, 2, 1, 3)
        outs.append(full.reshape(MC, N, D))
    return np.concatenate(outs, axis=0)
